# revision 38
# baseline (speedup 1.0000x reference)
"""TransformerConv 2-layer GNN encoder on 8 Trainium2 NeuronCores (Bass/Tile).

v2 strategy (graph-partition parallel, bf16 tables, per-tile batching):
  - Nodes padded 50000 -> 50176 = 8 cores x 49 tiles x 128. Each core owns 49
    consecutive node tiles as TARGETS; edges assigned to the dst core, sorted
    by dst, packed into 128-edge chunks per tile (chunk counts equalized
    across cores so the SPMD program is identical).
  - Phase A (per layer): q for LOCAL tiles from x_localT (per-core input,
    SPMD-safe addressing); k|v for ALL tiles from x_fullT -> kv_tab
    [50176, 256] bf16. Host provides x transposed so no PE transposes needed.
  - Edge phase (per layer, per tile, batched over the tile's chunks):
      per chunk: ONE merged k|v indirect gather (512B rows, bf16);
      eps = ea@We on PE (4-chunk PSUM groups, single ACT evacuation);
      S one-hot [slot, c] built batched on DVE; ST = S^T via PE transpose
      (4-chunk PSUM groups); qg = ST^T@qtile on PE (q never gathered);
      batched DVE: kj=k+eps, vj=v+eps, prod=kj*qg, alpha=group-reduce,
      exp on ACT (straight into the rhs tile), vjw=vj*exp;
      segment softmax-sum via S^T@[vjw|exp] accumulated in PSUM per tile;
      fused divide + skip (PE) + lrelu; h stored transposed for layer 2.
  - One AllGather of hT (12.8MB bf16) between the layers.
Softmax: segment-max subtraction skipped (alphas are O(0.3); exact softmax
invariance) and the divide applied after summation - matches reference.
"""
import numpy as np

P = 128
N = 50000
NP_ = 50176
TILES = 392
NCORES = 8
TPC = TILES // NCORES          # 49 tiles per core
NLC = TPC * P                  # 6272 local nodes
NODE_DIM = 128
EDGE_DIM = 16
HID = 128
DSTREL_PAD = 200.0
EGRP = 4                       # chunks per PSUM staging group


# ----------------------------------------------------------------- host prep
def _prep(ei, ea):
    import ml_dtypes
    src = np.asarray(ei[0], dtype=np.int64)
    dst = np.asarray(ei[1], dtype=np.int64)
    ea = np.asarray(ea, dtype=np.float32)

    order = np.argsort(dst, kind="stable")
    src_s, dst_s, ea_s = src[order], dst[order], ea[order]

    tile_of = dst_s // P
    cnt = np.bincount(tile_of, minlength=TILES)
    C = (cnt + P - 1) // P
    Cloc = np.maximum(C.reshape(NCORES, TPC).max(axis=0), 1)   # [TPC]
    NCH = int(Cloc.sum())
    off = np.zeros(TPC, dtype=np.int64)
    off[1:] = np.cumsum(Cloc)[:-1]

    tile_starts = np.searchsorted(tile_of, np.arange(TILES))
    tile_ends = np.searchsorted(tile_of, np.arange(TILES), side="right")
    cores = []
    for c in range(NCORES):
        nslot = NCH * P
        src_sl = np.zeros(nslot, dtype=np.int32)
        drel_sl = np.full(nslot, DSTREL_PAD, dtype=np.float32)
        ea_sl = np.zeros((nslot, EDGE_DIM), dtype=np.float32)
        for tl in range(TPC):
            tg = c * TPC + tl
            a, b = tile_starts[tg], tile_ends[tg]
            if b == a:
                continue
            s0 = off[tl] * P
            src_sl[s0:s0 + b - a] = src_s[a:b]
            drel_sl[s0:s0 + b - a] = (dst_s[a:b] - tg * P).astype(np.float32)
            ea_sl[s0:s0 + b - a] = ea_s[a:b]
        cores.append(dict(
            srcT=np.ascontiguousarray(src_sl.reshape(NCH, P).T),
            dstrelT=np.ascontiguousarray(drel_sl.reshape(NCH, P).T),
            eaT=np.ascontiguousarray(ea_sl.T).astype(ml_dtypes.bfloat16),
        ))
    return cores, Cloc, off, NCH


# ------------------------------------------------------- walrus wait legalize
def _legalize_waits(nc):
    import concourse.mybir as mybir
    k = 0
    for bb in nc.main_func.blocks:
        il = bb.instructions
        new = []
        for ins in il:
            si = ins.sync_info
            if si is not None and len(si.on_wait) > 1:
                waits = list(si.on_wait)
                for w in waits[:-1]:
                    nop = mybir.InstNoOp(name=f"wn{k}-{ins.name}", ins=[], outs=[])
                    k += 1
                    nop.engine = ins.engine
                    nop.sync_info = mybir.SyncInfo(on_wait=[w], on_update=[])
                    new.append(nop)
                ins.sync_info = mybir.SyncInfo(on_wait=[waits[-1]],
                                               on_update=list(si.on_update))
            new.append(ins)
        il[:] = new


# ------------------------------------------------------------- device program
def _build(Cloc, off, NCH):
    import concourse.bass as bass
    import concourse.mybir as mybir
    import concourse.tile as tile
    f32 = mybir.dt.float32
    bf16 = mybir.dt.bfloat16
    i32 = mybir.dt.int32
    Alu = mybir.AluOpType
    Act = mybir.ActivationFunctionType

    nc = bass.Bass()
    dp = nc.declare_dram_parameter
    x_fullT = dp("x_fullT", [NODE_DIM, NP_], bf16, isOutput=False)
    x_localT = dp("x_localT", [NODE_DIM, NLC], bf16, isOutput=False)
    srcT = dp("srcT", [P, NCH], i32, isOutput=False)
    dstrelT = dp("dstrelT", [P, NCH], f32, isOutput=False)
    eaT = dp("eaT", [EDGE_DIM, NCH * P], bf16, isOutput=False)
    iota_in = dp("iota_in", [P, P], f32, isOutput=False)
    ident_in = dp("ident_in", [P, P], bf16, isOutput=False)
    Wqkv1 = dp("Wqkv1", [NODE_DIM, 3 * HID], bf16, isOutput=False)
    We1 = dp("We1", [EDGE_DIM, HID], bf16, isOutput=False)
    Ws1 = dp("Ws1", [NODE_DIM, HID], bf16, isOutput=False)
    Wqkv2 = dp("Wqkv2", [HID, 3 * HID], bf16, isOutput=False)
    We2 = dp("We2", [EDGE_DIM, HID], bf16, isOutput=False)
    Ws2 = dp("Ws2", [HID, HID], bf16, isOutput=False)
    out = dp("out", [NLC, HID], f32, isOutput=True)

    # kv rows stored as raw 512B; declared f32 so the indirect gather walks
    # 128 4-byte elements per row instead of 256 2-byte ones
    kv_tab = nc.dram_tensor("kv_tab", [NP_, HID], f32, kind="Internal")
    q_loc = nc.dram_tensor("q_loc", [NLC, HID], bf16, kind="Internal")
    hT_loc = nc.dram_tensor("hT_loc", [HID, NLC], bf16, kind="Internal")
    kv2_loc = nc.dram_tensor("kv2_loc", [NLC, HID], f32, kind="Internal")

    # ---------------- phase A: q for local tiles, k|v table for all tiles
    # 4-tile batches: one load DMA, 4 matmuls, one evacuation, one store DMA.
    BA = 4

    def _qkv_loops(tc, cst, pool, psp, wt, qsrcT, kvsrc_slice):
        for i0 in range(0, TPC, BA):
            nb = min(BA, TPC - i0)
            xt = pool.tile([P, BA * P], bf16, tag="xl")
            nc.sync.dma_start(out=xt[:, 0:nb * P],
                              in_=qsrcT(i0, nb))
            ps = psp.tile([P, BA * HID], f32, space="PSUM", tag="pq")
            for j in range(nb):
                nc.tensor.matmul(out=ps[:, j * HID:(j + 1) * HID],
                                 lhsT=xt[:, j * P:(j + 1) * P],
                                 rhs=wt[:, 0:HID], start=True, stop=True)
            ev = pool.tile([P, BA * HID], bf16, tag="evq")
            nc.scalar.activation(out=ev[:, 0:nb * HID], in_=ps[:, 0:nb * HID],
                                 func=Act.Copy)
            nc.sync.dma_start(
                out=q_loc[i0 * P:(i0 + nb) * P, :].rearrange(
                    "(j p) d -> p j d", j=nb),
                in_=ev[:, 0:nb * HID].rearrange("p (j d) -> p j d", j=nb))
        for i0 in range(0, TILES, BA):
            nb = min(BA, TILES - i0)
            xt = pool.tile([P, BA * P], bf16, tag="xf")
            nc.sync.dma_start(out=xt[:, 0:nb * P], in_=kvsrc_slice(i0, nb))
            ps = psp.tile([P, BA * 2 * HID], f32, space="PSUM", tag="pkv")
            for j in range(nb):
                nc.tensor.matmul(out=ps[:, j * 2 * HID:(j + 1) * 2 * HID],
                                 lhsT=xt[:, j * P:(j + 1) * P],
                                 rhs=wt[:, HID:3 * HID], start=True, stop=True)
            ev = pool.tile([P, BA * 2 * HID], bf16, tag="evkv")
            nc.scalar.activation(out=ev[:, 0:nb * 2 * HID],
                                 in_=ps[:, 0:nb * 2 * HID], func=Act.Copy)
            nc.sync.dma_start(
                out=kv_tab[i0 * P:(i0 + nb) * P, :].rearrange(
                    "(j p) d -> p j d", j=nb),
                in_=ev[:].bitcast(f32)[:, 0:nb * HID].rearrange(
                    "p (j d) -> p j d", j=nb))

    def qkv_phase(locT, fullT, wqkv):
        with tile.TileContext(nc) as tc:
            with tc.tile_pool(name="qa_c", bufs=1) as cst, \
                 tc.tile_pool(name="qa_s", bufs=3) as pool, \
                 tc.tile_pool(name="qa_p", bufs=2, space="PSUM") as psp:
                wt = cst.tile([NODE_DIM, 3 * HID], bf16)
                nc.sync.dma_start(out=wt[:], in_=wqkv[:])
                _qkv_loops(tc, cst, pool, psp, wt,
                           lambda i0, nb: locT[:, i0 * P:(i0 + nb) * P],
                           lambda i0, nb: fullT[:, i0 * P:(i0 + nb) * P])

    # ---------------- edge phase
    def edge_phase(we, ws, xlocT, heads, relu, allgather_first):
        D = HID // heads
        scale = 1.0 / float(np.sqrt(D))
        W = HID + heads
        NMAX = int(Cloc.max())
        SB = 4                         # tiles per batched slab load
        NMAX4 = int(max(Cloc[t0:t0 + SB].sum()
                        for t0 in range(0, TPC, SB)))
        with tile.TileContext(nc) as tc:
            with tc.tile_pool(name="eg_c", bufs=1) as cst, \
                 tc.tile_pool(name="eg_sl", bufs=4) as slp, \
                 tc.tile_pool(name="eg_g", bufs=5) as gp, \
                 tc.tile_pool(name="eg_w", bufs=2) as wp, \
                 tc.tile_pool(name="eg_n", bufs=3) as npool, \
                 tc.tile_pool(name="eg_ps", bufs=4, space="PSUM") as pstage, \
                 tc.tile_pool(name="eg_pt", bufs=2, space="PSUM") as pst, \
                 tc.tile_pool(name="eg_pa", bufs=2, space="PSUM") as psacc:
                iota_f = cst.tile([P, P], f32)
                nc.sync.dma_start(out=iota_f[:], in_=iota_in[:])
                idt = cst.tile([P, P], bf16)
                nc.sync.dma_start(out=idt[:], in_=ident_in[:])
                if allgather_first:
                    nc.gpsimd.collective_compute(
                        "AllGather", Alu.bypass,
                        replica_groups=[list(range(NCORES))],
                        ins=[kv2_loc[:].opt()], outs=[kv_tab[:].opt()])
                wet = cst.tile([EDGE_DIM, HID], bf16)
                nc.sync.dma_start(out=wet[:], in_=we[:])
                wst = cst.tile([HID, HID], bf16)
                nc.sync.dma_start(out=wst[:], in_=ws[:])
                wq2 = wkv2 = None
                if relu:
                    wq2 = cst.tile([HID, HID], bf16)
                    nc.sync.dma_start(out=wq2[:], in_=Wqkv2[:, 0:HID])
                    wkv2 = cst.tile([HID, 2 * HID], bf16)
                    nc.sync.dma_start(out=wkv2[:], in_=Wqkv2[:, HID:3 * HID])

                for tl in range(TPC):
                    nch = int(Cloc[tl])
                    c0 = int(off[tl])
                    FD = nch * P
                    ngr = (nch + EGRP - 1) // EGRP

                    if tl % SB == 0:
                        t0b = tl
                        nch4 = int(Cloc[t0b:t0b + SB].sum())
                        cb0 = int(off[t0b])
                        ssl4 = slp.tile([P, NMAX4], i32, tag="ssl")
                        nc.sync.dma_start(out=ssl4[:, 0:nch4],
                                          in_=srcT[:, cb0:cb0 + nch4])
                        dsl4 = slp.tile([P, NMAX4], f32, tag="dsl")
                        nc.sync.dma_start(out=dsl4[:, 0:nch4],
                                          in_=dstrelT[:, cb0:cb0 + nch4])
                        nt4 = min(SB, TPC - t0b)
                        qt4 = slp.tile([P, SB * HID], bf16, tag="qt")
                        nc.sync.dma_start(
                            out=qt4[:, 0:nt4 * HID].rearrange(
                                "p (j d) -> p j d", j=nt4),
                            in_=q_loc[t0b * P:(t0b + nt4) * P, :].rearrange(
                                "(j p) d -> p j d", j=nt4))
                        xsk4 = slp.tile([P, SB * P], bf16, tag="xsk")
                        nc.sync.dma_start(out=xsk4[:, 0:nt4 * P],
                                          in_=xlocT[:, t0b * P:(t0b + nt4) * P])
                    sb_off = int(off[tl]) - int(off[t0b])
                    ssl = ssl4[:, sb_off:sb_off + nch]
                    dsl = dsl4[:, sb_off:sb_off + nch]
                    qtile = qt4[:, (tl - t0b) * HID:(tl - t0b + 1) * HID]
                    xsk = xsk4[:, (tl - t0b) * P:(tl - t0b + 1) * P]
                    ea_all = slp.tile([EDGE_DIM, NMAX * P], bf16, tag="ea")
                    nc.sync.dma_start(out=ea_all[:, 0:FD],
                                      in_=eaT[:, c0 * P:(c0 + nch) * P])

                    # merged k|v gathers, one per chunk (f32-typed raw rows)
                    kvg_r = gp.tile([P, NMAX * HID], f32, tag="kvg")
                    for k in range(nch):
                        nc.gpsimd.indirect_dma_start(
                            out=kvg_r[:, k * HID:(k + 1) * HID],
                            out_offset=None, in_=kv_tab[:],
                            in_offset=bass.IndirectOffsetOnAxis(
                                ap=ssl[:, k:k + 1], axis=0))
                    kvg = kvg_r[:].bitcast(bf16)

                    # S one-hot [slot, (j, c)] batched
                    S_all = wp.tile([P, NMAX * P], bf16, tag="S")
                    nc.vector.tensor_tensor(
                        out=S_all[:, 0:FD].rearrange("p (j c) -> p j c", j=nch),
                        in0=dsl.unsqueeze(2).to_broadcast([P, nch, P]),
                        in1=iota_f[:].unsqueeze(1).to_broadcast([P, nch, P]),
                        op=Alu.is_equal)

                    # eps / ST / qg staged through PSUM in EGRP-chunk groups
                    eps_sb = wp.tile([P, NMAX * HID], bf16, tag="eps")
                    st_sb = wp.tile([P, NMAX * P], bf16, tag="st")
                    qg_sb = wp.tile([P, NMAX * HID], bf16, tag="qg")
                    for g in range(ngr):
                        k0, k1 = g * EGRP, min(nch, (g + 1) * EGRP)
                        nk = k1 - k0
                        pe = pstage.tile([P, EGRP * HID], f32, space="PSUM",
                                         tag="stage")
                        for k in range(k0, k1):
                            j = k - k0
                            nc.tensor.matmul(
                                out=pe[:, j * HID:(j + 1) * HID],
                                lhsT=ea_all[:, k * P:(k + 1) * P],
                                rhs=wet[:], start=True, stop=True)
                        nc.scalar.activation(out=eps_sb[:, k0 * HID:k1 * HID],
                                             in_=pe[:, 0:nk * HID], func=Act.Copy)
                        pt = pst.tile([P, EGRP * P], bf16, space="PSUM",
                                      tag="staget")
                        for k in range(k0, k1):
                            j = k - k0
                            nc.tensor.transpose(
                                out=pt[:, j * P:(j + 1) * P],
                                in_=S_all[:, k * P:(k + 1) * P], identity=idt[:])
                        nc.scalar.activation(out=st_sb[:, k0 * P:k1 * P],
                                             in_=pt[:, 0:nk * P], func=Act.Copy)
                        pq = pstage.tile([P, EGRP * HID], f32, space="PSUM",
                                         tag="stage")
                        for k in range(k0, k1):
                            j = k - k0
                            nc.tensor.matmul(
                                out=pq[:, j * HID:(j + 1) * HID],
                                lhsT=st_sb[:, k * P:(k + 1) * P],
                                rhs=qtile, start=True, stop=True)
                        nc.scalar.activation(out=qg_sb[:, k0 * HID:k1 * HID],
                                             in_=pq[:, 0:nk * HID], func=Act.Copy)

                    # batched DVE: kj, vj, prod, alpha
                    kj = wp.tile([P, NMAX * HID], bf16, tag="kj")
                    nc.vector.tensor_tensor(
                        out=kj[:, 0:FD].rearrange("p (j d) -> p j d", j=nch),
                        in0=kvg[:, 0:nch * 2 * HID].rearrange(
                            "p (j d) -> p j d", j=nch)[:, :, 0:HID],
                        in1=eps_sb[:, 0:FD].rearrange("p (j d) -> p j d", j=nch),
                        op=Alu.add)
                    vj = wp.tile([P, NMAX * HID], bf16, tag="vj")
                    nc.vector.tensor_tensor(
                        out=vj[:, 0:FD].rearrange("p (j d) -> p j d", j=nch),
                        in0=kvg[:, 0:nch * 2 * HID].rearrange(
                            "p (j d) -> p j d", j=nch)[:, :, HID:2 * HID],
                        in1=eps_sb[:, 0:FD].rearrange("p (j d) -> p j d", j=nch),
                        op=Alu.add)
                    prod = wp.tile([P, NMAX * HID], bf16, tag="prod")
                    nc.vector.tensor_tensor(out=prod[:, 0:FD], in0=kj[:, 0:FD],
                                            in1=qg_sb[:, 0:FD], op=Alu.mult)
                    alpha = wp.tile([P, NMAX * 8], bf16, tag="alpha")
                    with nc.allow_low_precision(reason="attention logits are O(1); "
                                                "bf16 sum of 16 terms is fine"):
                        nc.vector.tensor_reduce(
                            out=alpha[:, 0:nch * heads],
                            in_=prod[:, 0:FD].rearrange("p (g d) -> p g d", d=D),
                            axis=mybir.AxisListType.X, op=Alu.add)

                    # rhs = [vj*exp | exp]
                    rhs = wp.tile([P, NMAX * W], bf16, tag="rhs")
                    rhs3 = rhs[:, 0:nch * W].rearrange("p (j w) -> p j w", j=nch)
                    nc.scalar.activation(
                        out=rhs3[:, :, HID:W],
                        in_=alpha[:, 0:nch * heads].rearrange(
                            "p (j h) -> p j h", j=nch),
                        func=Act.Exp, scale=scale)
                    nc.vector.tensor_tensor(
                        out=rhs3[:, :, 0:HID].rearrange(
                            "p j (h d) -> p j h d", h=heads),
                        in0=vj[:, 0:FD].rearrange(
                            "p (j h d) -> p j h d", j=nch, h=heads),
                        in1=rhs3[:, :, HID:W].unsqueeze(3).to_broadcast(
                            [P, nch, heads, D]),
                        op=Alu.mult)

                    # segment sum via one-hot matmul, accumulated per tile
                    acc = psacc.tile([P, W], f32, space="PSUM", tag="acc")
                    for k in range(nch):
                        nc.tensor.matmul(
                            out=acc[:], lhsT=S_all[:, k * P:(k + 1) * P],
                            rhs=rhs[:, k * W:(k + 1) * W],
                            start=(k == 0), stop=(k == nch - 1))

                    # ---- node update
                    sb_t = npool.tile([P, heads], f32, tag="sb")
                    nc.vector.tensor_scalar_add(out=sb_t[:],
                                                in0=acc[:, HID:W], scalar1=1e-16)
                    rinv = npool.tile([P, heads], f32, tag="rinv")
                    nc.vector.reciprocal(out=rinv[:], in_=sb_t[:])
                    attn = npool.tile([P, HID], f32, tag="attn")
                    nc.vector.tensor_tensor(
                        out=attn[:].rearrange("p (h d) -> p h d", h=heads),
                        in0=acc[:, 0:HID].rearrange("p (h d) -> p h d", h=heads),
                        in1=rinv[:].unsqueeze(2).to_broadcast([P, heads, D]),
                        op=Alu.mult)
                    skt = pstage.tile([P, EGRP * HID], f32, space="PSUM",
                                      tag="stage")
                    sk = skt[:, 0:HID]
                    nc.tensor.matmul(out=sk, lhsT=xsk, rhs=wst[:],
                                     start=True, stop=True)
                    ht = npool.tile([P, HID], f32, tag="ht")
                    nc.vector.tensor_tensor(out=ht[:], in0=attn[:], in1=sk,
                                            op=Alu.add)
                    if relu:
                        ht2 = npool.tile([P, HID], bf16, tag="ht2")
                        nc.scalar.activation(out=ht2[:], in_=ht[:], func=Act.Lrelu,
                                             alpha=0.01)
                        tpt = pst.tile([P, EGRP * P], bf16, space="PSUM",
                                       tag="staget")
                        tp = tpt[:, 0:P]
                        nc.tensor.transpose(out=tp, in_=ht2[:], identity=idt[:])
                        hTt = npool.tile([P, P], bf16, tag="hTt")
                        nc.scalar.activation(out=hTt[:], in_=tp, func=Act.Copy)
                        nc.sync.dma_start(out=hT_loc[:, tl * P:(tl + 1) * P],
                                              in_=hTt[:])
                        # fused layer-2 q|k|v for this tile (hides the whole
                        # layer-2 phase A; kv2 gets AllGathered between layers)
                        q2t = pstage.tile([P, EGRP * HID], f32, space="PSUM",
                                          tag="stage")
                        nc.tensor.matmul(out=q2t[:, 0:HID], lhsT=hTt[:],
                                         rhs=wq2[:], start=True, stop=True)
                        q2e = npool.tile([P, HID], bf16, tag="q2e")
                        nc.scalar.activation(out=q2e[:], in_=q2t[:, 0:HID],
                                             func=Act.Copy)
                        nc.sync.dma_start(out=q_loc[tl * P:(tl + 1) * P, :],
                                              in_=q2e[:])
                        kv2t = pstage.tile([P, EGRP * HID], f32, space="PSUM",
                                           tag="stage")
                        nc.tensor.matmul(out=kv2t[:, 0:2 * HID], lhsT=hTt[:],
                                         rhs=wkv2[:], start=True, stop=True)
                        kv2e = npool.tile([P, 2 * HID], bf16, tag="kv2e")
                        nc.scalar.activation(out=kv2e[:], in_=kv2t[:, 0:2 * HID],
                                             func=Act.Copy)
                        nc.sync.dma_start(
                            out=kv2_loc[tl * P:(tl + 1) * P, :],
                            in_=kv2e[:].bitcast(f32))
                    else:
                        nc.sync.dma_start(out=out[tl * P:(tl + 1) * P, :],
                                            in_=ht[:])


    qkv_phase(x_localT, x_fullT, Wqkv1)
    edge_phase(We1, Ws1, x_localT, heads=8, relu=True, allgather_first=False)
    edge_phase(We2, Ws2, hT_loc, heads=1, relu=False, allgather_first=True)

    _legalize_waits(nc)
    return nc


_CACHE = {}


def kernel(x, ei, ea, Wq1, bq1, Wk1, bk1, Wv1, bv1, We1, Ws1, bs1,
           Wq2, bq2, Wk2, bk2, Wv2, bv2, We2, Ws2, bs2):
    import ml_dtypes
    from concourse.bass_utils import run_bass_kernel_spmd
    bf = ml_dtypes.bfloat16

    for b in (bq1, bk1, bv1, bs1, bq2, bk2, bv2, bs2):
        assert not np.any(np.asarray(b)), "nonzero biases not supported"

    x = np.asarray(x, np.float32)
    x_pad = np.zeros((NP_, NODE_DIM), np.float32)
    x_pad[:N] = x
    x_fullT = np.ascontiguousarray(x_pad.T).astype(bf)
    cores, Cloc, off, NCH = _prep(np.asarray(ei), np.asarray(ea))

    key = (NCH, tuple(Cloc))
    if key not in _CACHE:
        _CACHE[key] = _build(Cloc, off, NCH)
    nc = _CACHE[key]

    def cat3(a, b, c):
        return np.ascontiguousarray(np.concatenate(
            [np.asarray(a, np.float32), np.asarray(b, np.float32),
             np.asarray(c, np.float32)], axis=1)).astype(bf)

    Wqkv1 = cat3(Wq1, Wk1, Wv1)
    Wqkv2 = cat3(Wq2, Wk2, Wv2)
    iota_in = np.tile(np.arange(P, dtype=np.float32)[None, :], (P, 1))
    ident_in = np.eye(P, dtype=np.float32).astype(bf)

    in_maps = []
    for c in range(NCORES):
        pc = cores[c]
        in_maps.append({
            "x_fullT": x_fullT,
            "x_localT": np.ascontiguousarray(
                x_fullT[:, c * NLC:(c + 1) * NLC]),
            "srcT": pc["srcT"], "dstrelT": pc["dstrelT"], "eaT": pc["eaT"],
            "iota_in": iota_in, "ident_in": ident_in,
            "Wqkv1": Wqkv1, "We1": np.asarray(We1, np.float32).astype(bf),
            "Ws1": np.asarray(Ws1, np.float32).astype(bf),
            "Wqkv2": Wqkv2, "We2": np.asarray(We2, np.float32).astype(bf),
            "Ws2": np.asarray(Ws2, np.float32).astype(bf),
        })
    res = run_bass_kernel_spmd(nc, in_maps, list(range(NCORES)))
    global LAST_RESULT
    LAST_RESULT = res
    out = np.concatenate([res.results[c]["out"] for c in range(NCORES)], axis=0)
    return np.ascontiguousarray(out[:N])


LAST_RESULT = None


# revision 39
# speedup vs baseline: 1.0046x; 1.0046x over previous
"""TransformerConv 2-layer GNN encoder on 8 Trainium2 NeuronCores (Bass/Tile).

Graph-partition parallel, bf16 tables, per-tile batching (8.49ms -> 3.23ms):
  - Nodes padded 50000 -> 50176 = 8 cores x 49 tiles x 128. Each core owns 49
    consecutive node tiles as TARGETS; edges assigned to the dst core, sorted
    by dst, packed into 128-edge chunks per tile (chunk counts equalized
    across cores so the SPMD program is identical).
  - Phase A (layer 1): q for LOCAL tiles from x_localT (per-core input,
    SPMD-safe addressing); k|v for ALL tiles from x_fullT -> kv_tab (bf16
    rows stored via an f32 bitcast view). Host pre-transposes x, so phase A
    needs no PE transposes; 4-tile batches share one load/evac/store.
  - Edge phase (per layer, per tile, batched over the tile's chunks):
      per chunk ONE merged k|v indirect gather (512B bf16 rows) - the
      critical resource: ~1.4us of GPSIMD/SWDGE time per gather;
      eps = ea@We on PE (4-chunk PSUM groups, single ACT evacuation);
      S one-hot [slot, c] built batched on DVE; ST = S^T via PE transpose;
      qg = ST^T@qtile on PE (q is never gathered - dst is tile-local);
      batched DVE: kj=k+eps, vj=v+eps, prod=kj*qg, alpha=group-reduce (bf16);
      exp on ACT written straight into the rhs tile; vjw=vj*exp;
      segment softmax-sum via S^T@[vjw|exp] accumulated in PSUM per tile;
      fused divide + skip matmul + lrelu. Layer 1's node update also computes
      hT (transposed), q2 and kv2 for layer 2 in place (hides layer-2 phase A).
  - One AllGather of kv2 (25.7MB out, bf16) issued at the START of the
    layer-2 edge context so slab loads/S-builds overlap it.
Softmax: segment-max subtraction skipped (alphas are O(0.3); exact softmax
invariance) and the divide applied after summation - matches reference.
Known floor: 1666 indirect gathers x ~1.4us on the GPSIMD queue (~2.3ms);
the batched/ext-isa gather paths (multi-offset InstDMACopy, dma_gather) are
broken or hang under this runtime, so per-chunk gathers are the minimum.
"""
import numpy as np

P = 128
N = 50000
NP_ = 50176
TILES = 392
NCORES = 8
TPC = TILES // NCORES          # 49 tiles per core
NLC = TPC * P                  # 6272 local nodes
NODE_DIM = 128
EDGE_DIM = 16
HID = 128
DSTREL_PAD = 200.0
EGRP = 4                       # chunks per PSUM staging group


# ----------------------------------------------------------------- host prep
def _prep(ei, ea):
    import ml_dtypes
    src = np.asarray(ei[0], dtype=np.int64)
    dst = np.asarray(ei[1], dtype=np.int64)
    ea = np.asarray(ea, dtype=np.float32)

    order = np.argsort(dst, kind="stable")
    src_s, dst_s, ea_s = src[order], dst[order], ea[order]

    tile_of = dst_s // P
    cnt = np.bincount(tile_of, minlength=TILES)
    C = (cnt + P - 1) // P
    Cloc = np.maximum(C.reshape(NCORES, TPC).max(axis=0), 1)   # [TPC]
    NCH = int(Cloc.sum())
    off = np.zeros(TPC, dtype=np.int64)
    off[1:] = np.cumsum(Cloc)[:-1]

    tile_starts = np.searchsorted(tile_of, np.arange(TILES))
    tile_ends = np.searchsorted(tile_of, np.arange(TILES), side="right")
    cores = []
    for c in range(NCORES):
        nslot = NCH * P
        src_sl = np.zeros(nslot, dtype=np.int32)
        drel_sl = np.full(nslot, DSTREL_PAD, dtype=np.float32)
        ea_sl = np.zeros((nslot, EDGE_DIM), dtype=np.float32)
        for tl in range(TPC):
            tg = c * TPC + tl
            a, b = tile_starts[tg], tile_ends[tg]
            if b == a:
                continue
            s0 = off[tl] * P
            src_sl[s0:s0 + b - a] = src_s[a:b]
            drel_sl[s0:s0 + b - a] = (dst_s[a:b] - tg * P).astype(np.float32)
            ea_sl[s0:s0 + b - a] = ea_s[a:b]
        cores.append(dict(
            srcT=np.ascontiguousarray(src_sl.reshape(NCH, P).T),
            dstrelT=np.ascontiguousarray(drel_sl.reshape(NCH, P).T),
            eaT=np.ascontiguousarray(ea_sl.T).astype(ml_dtypes.bfloat16),
        ))
    return cores, Cloc, off, NCH


# ------------------------------------------------------- walrus wait legalize
def _legalize_waits(nc):
    import concourse.mybir as mybir
    k = 0
    for bb in nc.main_func.blocks:
        il = bb.instructions
        new = []
        for ins in il:
            si = ins.sync_info
            if si is not None and len(si.on_wait) > 1:
                waits = list(si.on_wait)
                for w in waits[:-1]:
                    nop = mybir.InstNoOp(name=f"wn{k}-{ins.name}", ins=[], outs=[])
                    k += 1
                    nop.engine = ins.engine
                    nop.sync_info = mybir.SyncInfo(on_wait=[w], on_update=[])
                    new.append(nop)
                ins.sync_info = mybir.SyncInfo(on_wait=[waits[-1]],
                                               on_update=list(si.on_update))
            new.append(ins)
        il[:] = new


# ------------------------------------------------------------- device program
def _build(Cloc, off, NCH):
    import concourse.bass as bass
    import concourse.mybir as mybir
    import concourse.tile as tile
    f32 = mybir.dt.float32
    bf16 = mybir.dt.bfloat16
    i32 = mybir.dt.int32
    Alu = mybir.AluOpType
    Act = mybir.ActivationFunctionType

    nc = bass.Bass()
    dp = nc.declare_dram_parameter
    x_fullT = dp("x_fullT", [NODE_DIM, NP_], bf16, isOutput=False)
    x_localT = dp("x_localT", [NODE_DIM, NLC], bf16, isOutput=False)
    srcT = dp("srcT", [P, NCH], i32, isOutput=False)
    dstrelT = dp("dstrelT", [P, NCH], f32, isOutput=False)
    eaT = dp("eaT", [EDGE_DIM, NCH * P], bf16, isOutput=False)
    iota_in = dp("iota_in", [P, P], f32, isOutput=False)
    ident_in = dp("ident_in", [P, P], bf16, isOutput=False)
    Wqkv1 = dp("Wqkv1", [NODE_DIM, 3 * HID], bf16, isOutput=False)
    We1 = dp("We1", [EDGE_DIM, HID], bf16, isOutput=False)
    Ws1 = dp("Ws1", [NODE_DIM, HID], bf16, isOutput=False)
    Wqkv2 = dp("Wqkv2", [HID, 3 * HID], bf16, isOutput=False)
    We2 = dp("We2", [EDGE_DIM, HID], bf16, isOutput=False)
    Ws2 = dp("Ws2", [HID, HID], bf16, isOutput=False)
    out = dp("out", [NLC, HID], f32, isOutput=True)

    # kv rows stored as raw 512B; declared f32 so the indirect gather walks
    # 128 4-byte elements per row instead of 256 2-byte ones
    kv_tab = nc.dram_tensor("kv_tab", [NP_, HID], f32, kind="Internal")
    q_loc = nc.dram_tensor("q_loc", [NLC, HID], bf16, kind="Internal")
    hT_loc = nc.dram_tensor("hT_loc", [HID, NLC], bf16, kind="Internal")
    kv2_loc = nc.dram_tensor("kv2_loc", [NLC, HID], f32, kind="Internal")

    # ---------------- phase A: q for local tiles, k|v table for all tiles
    # 4-tile batches: one load DMA, 4 matmuls, one evacuation, one store DMA.
    BA = 4

    def _qkv_loops(tc, cst, pool, psp, wt, qsrcT, kvsrc_slice):
        for i0 in range(0, TPC, BA):
            nb = min(BA, TPC - i0)
            xt = pool.tile([P, BA * P], bf16, tag="xl")
            nc.sync.dma_start(out=xt[:, 0:nb * P],
                              in_=qsrcT(i0, nb))
            ps = psp.tile([P, BA * HID], f32, space="PSUM", tag="pq")
            for j in range(nb):
                nc.tensor.matmul(out=ps[:, j * HID:(j + 1) * HID],
                                 lhsT=xt[:, j * P:(j + 1) * P],
                                 rhs=wt[:, 0:HID], start=True, stop=True)
            ev = pool.tile([P, BA * HID], bf16, tag="evq")
            nc.scalar.activation(out=ev[:, 0:nb * HID], in_=ps[:, 0:nb * HID],
                                 func=Act.Copy)
            nc.sync.dma_start(
                out=q_loc[i0 * P:(i0 + nb) * P, :].rearrange(
                    "(j p) d -> p j d", j=nb),
                in_=ev[:, 0:nb * HID].rearrange("p (j d) -> p j d", j=nb))
        for i0 in range(0, TILES, BA):
            nb = min(BA, TILES - i0)
            xt = pool.tile([P, BA * P], bf16, tag="xf")
            nc.sync.dma_start(out=xt[:, 0:nb * P], in_=kvsrc_slice(i0, nb))
            ps = psp.tile([P, BA * 2 * HID], f32, space="PSUM", tag="pkv")
            for j in range(nb):
                nc.tensor.matmul(out=ps[:, j * 2 * HID:(j + 1) * 2 * HID],
                                 lhsT=xt[:, j * P:(j + 1) * P],
                                 rhs=wt[:, HID:3 * HID], start=True, stop=True)
            ev = pool.tile([P, BA * 2 * HID], bf16, tag="evkv")
            nc.scalar.activation(out=ev[:, 0:nb * 2 * HID],
                                 in_=ps[:, 0:nb * 2 * HID], func=Act.Copy)
            nc.sync.dma_start(
                out=kv_tab[i0 * P:(i0 + nb) * P, :].rearrange(
                    "(j p) d -> p j d", j=nb),
                in_=ev[:].bitcast(f32)[:, 0:nb * HID].rearrange(
                    "p (j d) -> p j d", j=nb))

    def qkv_phase(locT, fullT, wqkv):
        with tile.TileContext(nc) as tc:
            with tc.tile_pool(name="qa_c", bufs=1) as cst, \
                 tc.tile_pool(name="qa_s", bufs=3) as pool, \
                 tc.tile_pool(name="qa_p", bufs=2, space="PSUM") as psp:
                wt = cst.tile([NODE_DIM, 3 * HID], bf16)
                nc.sync.dma_start(out=wt[:], in_=wqkv[:])
                _qkv_loops(tc, cst, pool, psp, wt,
                           lambda i0, nb: locT[:, i0 * P:(i0 + nb) * P],
                           lambda i0, nb: fullT[:, i0 * P:(i0 + nb) * P])

    # ---------------- edge phase
    def edge_phase(we, ws, xlocT, heads, relu, allgather_first):
        D = HID // heads
        scale = 1.0 / float(np.sqrt(D))
        W = HID + heads
        NMAX = int(Cloc.max())
        SB = 4                         # tiles per batched slab load
        NMAX4 = int(max(Cloc[t0:t0 + SB].sum()
                        for t0 in range(0, TPC, SB)))
        with tile.TileContext(nc) as tc:
            with tc.tile_pool(name="eg_c", bufs=1) as cst, \
                 tc.tile_pool(name="eg_sl", bufs=3) as slp, \
                 tc.tile_pool(name="eg_g", bufs=4) as gp, \
                 tc.tile_pool(name="eg_w", bufs=2) as wp, \
                 tc.tile_pool(name="eg_n", bufs=2) as npool, \
                 tc.tile_pool(name="eg_ps", bufs=3, space="PSUM") as pstage, \
                 tc.tile_pool(name="eg_pt", bufs=2, space="PSUM") as pst, \
                 tc.tile_pool(name="eg_pa", bufs=2, space="PSUM") as psacc:
                iota_f = cst.tile([P, P], f32)
                nc.sync.dma_start(out=iota_f[:], in_=iota_in[:])
                idt = cst.tile([P, P], bf16)
                nc.sync.dma_start(out=idt[:], in_=ident_in[:])
                if allgather_first:
                    nc.gpsimd.collective_compute(
                        "AllGather", Alu.bypass,
                        replica_groups=[list(range(NCORES))],
                        ins=[kv2_loc[:].opt()], outs=[kv_tab[:].opt()])
                wet = cst.tile([EDGE_DIM, HID], bf16)
                nc.sync.dma_start(out=wet[:], in_=we[:])
                wst = cst.tile([HID, HID], bf16)
                nc.sync.dma_start(out=wst[:], in_=ws[:])
                wq2 = wkv2 = None
                if relu:
                    wq2 = cst.tile([HID, HID], bf16)
                    nc.sync.dma_start(out=wq2[:], in_=Wqkv2[:, 0:HID])
                    wkv2 = cst.tile([HID, 2 * HID], bf16)
                    nc.sync.dma_start(out=wkv2[:], in_=Wqkv2[:, HID:3 * HID])

                for tl in range(TPC):
                    nch = int(Cloc[tl])
                    c0 = int(off[tl])
                    FD = nch * P
                    ngr = (nch + EGRP - 1) // EGRP

                    if tl % SB == 0:
                        t0b = tl
                        nch4 = int(Cloc[t0b:t0b + SB].sum())
                        cb0 = int(off[t0b])
                        ssl4 = slp.tile([P, NMAX4], i32, tag="ssl")
                        nc.sync.dma_start(out=ssl4[:, 0:nch4],
                                          in_=srcT[:, cb0:cb0 + nch4])
                        dsl4 = slp.tile([P, NMAX4], f32, tag="dsl")
                        nc.sync.dma_start(out=dsl4[:, 0:nch4],
                                          in_=dstrelT[:, cb0:cb0 + nch4])
                        nt4 = min(SB, TPC - t0b)
                        qt4 = slp.tile([P, SB * HID], bf16, tag="qt")
                        nc.sync.dma_start(
                            out=qt4[:, 0:nt4 * HID].rearrange(
                                "p (j d) -> p j d", j=nt4),
                            in_=q_loc[t0b * P:(t0b + nt4) * P, :].rearrange(
                                "(j p) d -> p j d", j=nt4))
                        xsk4 = slp.tile([P, SB * P], bf16, tag="xsk")
                        nc.sync.dma_start(out=xsk4[:, 0:nt4 * P],
                                          in_=xlocT[:, t0b * P:(t0b + nt4) * P])
                    sb_off = int(off[tl]) - int(off[t0b])
                    ssl = ssl4[:, sb_off:sb_off + nch]
                    dsl = dsl4[:, sb_off:sb_off + nch]
                    qtile = qt4[:, (tl - t0b) * HID:(tl - t0b + 1) * HID]
                    xsk = xsk4[:, (tl - t0b) * P:(tl - t0b + 1) * P]
                    ea_all = slp.tile([EDGE_DIM, NMAX * P], bf16, tag="ea")
                    nc.sync.dma_start(out=ea_all[:, 0:FD],
                                      in_=eaT[:, c0 * P:(c0 + nch) * P])

                    # merged k|v gathers, one per chunk (f32-typed raw rows)
                    kvg_r = gp.tile([P, NMAX * HID], f32, tag="kvg")
                    for k in range(nch):
                        nc.gpsimd.indirect_dma_start(
                            out=kvg_r[:, k * HID:(k + 1) * HID],
                            out_offset=None, in_=kv_tab[:],
                            in_offset=bass.IndirectOffsetOnAxis(
                                ap=ssl[:, k:k + 1], axis=0))
                    kvg = kvg_r[:].bitcast(bf16)

                    # S one-hot [slot, (j, c)] batched
                    S_all = wp.tile([P, NMAX * P], bf16, tag="S")
                    nc.vector.tensor_tensor(
                        out=S_all[:, 0:FD].rearrange("p (j c) -> p j c", j=nch),
                        in0=dsl.unsqueeze(2).to_broadcast([P, nch, P]),
                        in1=iota_f[:].unsqueeze(1).to_broadcast([P, nch, P]),
                        op=Alu.is_equal)

                    # eps / ST / qg staged through PSUM in EGRP-chunk groups
                    eps_sb = wp.tile([P, NMAX * HID], bf16, tag="eps")
                    st_sb = wp.tile([P, NMAX * P], bf16, tag="st")
                    qg_sb = wp.tile([P, NMAX * HID], bf16, tag="qg")
                    for g in range(ngr):
                        k0, k1 = g * EGRP, min(nch, (g + 1) * EGRP)
                        nk = k1 - k0
                        pe = pstage.tile([P, EGRP * HID], f32, space="PSUM",
                                         tag="stage")
                        for k in range(k0, k1):
                            j = k - k0
                            nc.tensor.matmul(
                                out=pe[:, j * HID:(j + 1) * HID],
                                lhsT=ea_all[:, k * P:(k + 1) * P],
                                rhs=wet[:], start=True, stop=True)
                        nc.scalar.activation(out=eps_sb[:, k0 * HID:k1 * HID],
                                             in_=pe[:, 0:nk * HID], func=Act.Copy)
                        pt = pst.tile([P, EGRP * P], bf16, space="PSUM",
                                      tag="staget")
                        for k in range(k0, k1):
                            j = k - k0
                            nc.tensor.transpose(
                                out=pt[:, j * P:(j + 1) * P],
                                in_=S_all[:, k * P:(k + 1) * P], identity=idt[:])
                        nc.scalar.activation(out=st_sb[:, k0 * P:k1 * P],
                                             in_=pt[:, 0:nk * P], func=Act.Copy)
                        pq = pstage.tile([P, EGRP * HID], f32, space="PSUM",
                                         tag="stage")
                        for k in range(k0, k1):
                            j = k - k0
                            nc.tensor.matmul(
                                out=pq[:, j * HID:(j + 1) * HID],
                                lhsT=st_sb[:, k * P:(k + 1) * P],
                                rhs=qtile, start=True, stop=True)
                        nc.scalar.activation(out=qg_sb[:, k0 * HID:k1 * HID],
                                             in_=pq[:, 0:nk * HID], func=Act.Copy)

                    # batched DVE: kj, vj, prod, alpha
                    kj = wp.tile([P, NMAX * HID], bf16, tag="kj")
                    nc.vector.tensor_tensor(
                        out=kj[:, 0:FD].rearrange("p (j d) -> p j d", j=nch),
                        in0=kvg[:, 0:nch * 2 * HID].rearrange(
                            "p (j d) -> p j d", j=nch)[:, :, 0:HID],
                        in1=eps_sb[:, 0:FD].rearrange("p (j d) -> p j d", j=nch),
                        op=Alu.add)
                    vj = wp.tile([P, NMAX * HID], bf16, tag="vj")
                    nc.vector.tensor_tensor(
                        out=vj[:, 0:FD].rearrange("p (j d) -> p j d", j=nch),
                        in0=kvg[:, 0:nch * 2 * HID].rearrange(
                            "p (j d) -> p j d", j=nch)[:, :, HID:2 * HID],
                        in1=eps_sb[:, 0:FD].rearrange("p (j d) -> p j d", j=nch),
                        op=Alu.add)
                    prod = wp.tile([P, NMAX * HID], bf16, tag="prod")
                    nc.vector.tensor_tensor(out=prod[:, 0:FD], in0=kj[:, 0:FD],
                                            in1=qg_sb[:, 0:FD], op=Alu.mult)
                    alpha = wp.tile([P, NMAX * 8], bf16, tag="alpha")
                    with nc.allow_low_precision(reason="attention logits are O(1); "
                                                "bf16 sum of 16 terms is fine"):
                        nc.vector.tensor_reduce(
                            out=alpha[:, 0:nch * heads],
                            in_=prod[:, 0:FD].rearrange("p (g d) -> p g d", d=D),
                            axis=mybir.AxisListType.X, op=Alu.add)

                    # rhs = [vj*exp | exp]
                    rhs = wp.tile([P, NMAX * W], bf16, tag="rhs")
                    rhs3 = rhs[:, 0:nch * W].rearrange("p (j w) -> p j w", j=nch)
                    nc.scalar.activation(
                        out=rhs3[:, :, HID:W],
                        in_=alpha[:, 0:nch * heads].rearrange(
                            "p (j h) -> p j h", j=nch),
                        func=Act.Exp, scale=scale)
                    nc.vector.tensor_tensor(
                        out=rhs3[:, :, 0:HID].rearrange(
                            "p j (h d) -> p j h d", h=heads),
                        in0=vj[:, 0:FD].rearrange(
                            "p (j h d) -> p j h d", j=nch, h=heads),
                        in1=rhs3[:, :, HID:W].unsqueeze(3).to_broadcast(
                            [P, nch, heads, D]),
                        op=Alu.mult)

                    # segment sum via one-hot matmul, accumulated per tile
                    acc = psacc.tile([P, W], f32, space="PSUM", tag="acc")
                    for k in range(nch):
                        nc.tensor.matmul(
                            out=acc[:], lhsT=S_all[:, k * P:(k + 1) * P],
                            rhs=rhs[:, k * W:(k + 1) * W],
                            start=(k == 0), stop=(k == nch - 1))

                    # ---- node update
                    sb_t = npool.tile([P, heads], f32, tag="sb")
                    nc.vector.tensor_scalar_add(out=sb_t[:],
                                                in0=acc[:, HID:W], scalar1=1e-16)
                    rinv = npool.tile([P, heads], f32, tag="rinv")
                    nc.vector.reciprocal(out=rinv[:], in_=sb_t[:])
                    attn = npool.tile([P, HID], f32, tag="attn")
                    nc.vector.tensor_tensor(
                        out=attn[:].rearrange("p (h d) -> p h d", h=heads),
                        in0=acc[:, 0:HID].rearrange("p (h d) -> p h d", h=heads),
                        in1=rinv[:].unsqueeze(2).to_broadcast([P, heads, D]),
                        op=Alu.mult)
                    skt = pstage.tile([P, EGRP * HID], f32, space="PSUM",
                                      tag="stage")
                    sk = skt[:, 0:HID]
                    nc.tensor.matmul(out=sk, lhsT=xsk, rhs=wst[:],
                                     start=True, stop=True)
                    ht = npool.tile([P, HID], f32, tag="ht")
                    nc.vector.tensor_tensor(out=ht[:], in0=attn[:], in1=sk,
                                            op=Alu.add)
                    if relu:
                        ht2 = npool.tile([P, HID], bf16, tag="ht2")
                        nc.scalar.activation(out=ht2[:], in_=ht[:], func=Act.Lrelu,
                                             alpha=0.01)
                        tpt = pst.tile([P, EGRP * P], bf16, space="PSUM",
                                       tag="staget")
                        tp = tpt[:, 0:P]
                        nc.tensor.transpose(out=tp, in_=ht2[:], identity=idt[:])
                        hTt = npool.tile([P, P], bf16, tag="hTt")
                        nc.scalar.activation(out=hTt[:], in_=tp, func=Act.Copy)
                        nc.sync.dma_start(out=hT_loc[:, tl * P:(tl + 1) * P],
                                              in_=hTt[:])
                        # fused layer-2 q|k|v for this tile (hides the whole
                        # layer-2 phase A; kv2 gets AllGathered between layers)
                        q2t = pstage.tile([P, EGRP * HID], f32, space="PSUM",
                                          tag="stage")
                        nc.tensor.matmul(out=q2t[:, 0:HID], lhsT=hTt[:],
                                         rhs=wq2[:], start=True, stop=True)
                        q2e = npool.tile([P, HID], bf16, tag="q2e")
                        nc.scalar.activation(out=q2e[:], in_=q2t[:, 0:HID],
                                             func=Act.Copy)
                        nc.sync.dma_start(out=q_loc[tl * P:(tl + 1) * P, :],
                                              in_=q2e[:])
                        kv2t = pstage.tile([P, EGRP * HID], f32, space="PSUM",
                                           tag="stage")
                        nc.tensor.matmul(out=kv2t[:, 0:2 * HID], lhsT=hTt[:],
                                         rhs=wkv2[:], start=True, stop=True)
                        kv2e = npool.tile([P, 2 * HID], bf16, tag="kv2e")
                        nc.scalar.activation(out=kv2e[:], in_=kv2t[:, 0:2 * HID],
                                             func=Act.Copy)
                        nc.sync.dma_start(
                            out=kv2_loc[tl * P:(tl + 1) * P, :],
                            in_=kv2e[:].bitcast(f32))
                    else:
                        nc.sync.dma_start(out=out[tl * P:(tl + 1) * P, :],
                                            in_=ht[:])


    qkv_phase(x_localT, x_fullT, Wqkv1)
    edge_phase(We1, Ws1, x_localT, heads=8, relu=True, allgather_first=False)
    edge_phase(We2, Ws2, hT_loc, heads=1, relu=False, allgather_first=True)

    _legalize_waits(nc)
    return nc


_CACHE = {}


def kernel(x, ei, ea, Wq1, bq1, Wk1, bk1, Wv1, bv1, We1, Ws1, bs1,
           Wq2, bq2, Wk2, bk2, Wv2, bv2, We2, Ws2, bs2):
    import ml_dtypes
    from concourse.bass_utils import run_bass_kernel_spmd
    bf = ml_dtypes.bfloat16

    for b in (bq1, bk1, bv1, bs1, bq2, bk2, bv2, bs2):
        assert not np.any(np.asarray(b)), "nonzero biases not supported"

    x = np.asarray(x, np.float32)
    x_pad = np.zeros((NP_, NODE_DIM), np.float32)
    x_pad[:N] = x
    x_fullT = np.ascontiguousarray(x_pad.T).astype(bf)
    cores, Cloc, off, NCH = _prep(np.asarray(ei), np.asarray(ea))

    key = (NCH, tuple(Cloc))
    if key not in _CACHE:
        _CACHE[key] = _build(Cloc, off, NCH)
    nc = _CACHE[key]

    def cat3(a, b, c):
        return np.ascontiguousarray(np.concatenate(
            [np.asarray(a, np.float32), np.asarray(b, np.float32),
             np.asarray(c, np.float32)], axis=1)).astype(bf)

    Wqkv1 = cat3(Wq1, Wk1, Wv1)
    Wqkv2 = cat3(Wq2, Wk2, Wv2)
    iota_in = np.tile(np.arange(P, dtype=np.float32)[None, :], (P, 1))
    ident_in = np.eye(P, dtype=np.float32).astype(bf)

    in_maps = []
    for c in range(NCORES):
        pc = cores[c]
        in_maps.append({
            "x_fullT": x_fullT,
            "x_localT": np.ascontiguousarray(
                x_fullT[:, c * NLC:(c + 1) * NLC]),
            "srcT": pc["srcT"], "dstrelT": pc["dstrelT"], "eaT": pc["eaT"],
            "iota_in": iota_in, "ident_in": ident_in,
            "Wqkv1": Wqkv1, "We1": np.asarray(We1, np.float32).astype(bf),
            "Ws1": np.asarray(Ws1, np.float32).astype(bf),
            "Wqkv2": Wqkv2, "We2": np.asarray(We2, np.float32).astype(bf),
            "Ws2": np.asarray(Ws2, np.float32).astype(bf),
        })
    res = run_bass_kernel_spmd(nc, in_maps, list(range(NCORES)))
    global LAST_RESULT
    LAST_RESULT = res
    out = np.concatenate([res.results[c]["out"] for c in range(NCORES)], axis=0)
    return np.ascontiguousarray(out[:N])


LAST_RESULT = None


# revision 40
# speedup vs baseline: 1.0086x; 1.0040x over previous
"""TransformerConv 2-layer GNN encoder on 8 Trainium2 NeuronCores (Bass/Tile).

Graph-partition parallel, bf16 tables, per-tile batching (8.49ms -> 3.23ms):
  - Nodes padded 50000 -> 50176 = 8 cores x 49 tiles x 128. Each core owns 49
    consecutive node tiles as TARGETS; edges assigned to the dst core, sorted
    by dst, packed into 128-edge chunks per tile (chunk counts equalized
    across cores so the SPMD program is identical).
  - Phase A (layer 1): q for LOCAL tiles from x_localT (per-core input,
    SPMD-safe addressing); k|v for ALL tiles from x_fullT -> kv_tab (bf16
    rows stored via an f32 bitcast view). Host pre-transposes x, so phase A
    needs no PE transposes; 4-tile batches share one load/evac/store.
  - Edge phase (per layer, per tile, batched over the tile's chunks):
      per chunk ONE merged k|v indirect gather (512B bf16 rows) - the
      critical resource: ~1.4us of GPSIMD/SWDGE time per gather;
      eps = ea@We on PE (4-chunk PSUM groups, single ACT evacuation);
      S one-hot [slot, c] built batched on DVE; ST = S^T via PE transpose;
      qg = ST^T@qtile on PE (q is never gathered - dst is tile-local);
      batched DVE: kj=k+eps, vj=v+eps, prod=kj*qg, alpha=group-reduce (bf16);
      exp on ACT written straight into the rhs tile; vjw=vj*exp;
      segment softmax-sum via S^T@[vjw|exp] accumulated in PSUM per tile;
      fused divide + skip matmul + lrelu. Layer 1's node update also computes
      hT (transposed), q2 and kv2 for layer 2 in place (hides layer-2 phase A).
  - One AllGather of kv2 (25.7MB out, bf16) issued at the START of the
    layer-2 edge context so slab loads/S-builds overlap it.
Softmax: segment-max subtraction skipped (alphas are O(0.3); exact softmax
invariance) and the divide applied after summation - matches reference.
Known floor: 1666 indirect gathers x ~1.4us on the GPSIMD queue (~2.3ms);
the batched/ext-isa gather paths (multi-offset InstDMACopy, dma_gather) are
broken or hang under this runtime, so per-chunk gathers are the minimum.
"""
import numpy as np

P = 128
N = 50000
NP_ = 50176
TILES = 392
NCORES = 8
TPC = TILES // NCORES          # 49 tiles per core
NLC = TPC * P                  # 6272 local nodes
NODE_DIM = 128
EDGE_DIM = 16
HID = 128
DSTREL_PAD = 200.0
EGRP = 4                       # chunks per PSUM staging group


# ----------------------------------------------------------------- host prep
def _prep(ei, ea):
    import ml_dtypes
    src = np.asarray(ei[0], dtype=np.int64)
    dst = np.asarray(ei[1], dtype=np.int64)
    ea = np.asarray(ea, dtype=np.float32)

    order = np.argsort(dst, kind="stable")
    src_s, dst_s, ea_s = src[order], dst[order], ea[order]

    tile_of = dst_s // P
    cnt = np.bincount(tile_of, minlength=TILES)
    C = (cnt + P - 1) // P
    Cloc = np.maximum(C.reshape(NCORES, TPC).max(axis=0), 1)   # [TPC]
    NCH = int(Cloc.sum())
    off = np.zeros(TPC, dtype=np.int64)
    off[1:] = np.cumsum(Cloc)[:-1]

    tile_starts = np.searchsorted(tile_of, np.arange(TILES))
    tile_ends = np.searchsorted(tile_of, np.arange(TILES), side="right")
    cores = []
    for c in range(NCORES):
        nslot = NCH * P
        src_sl = np.zeros(nslot, dtype=np.int32)
        drel_sl = np.full(nslot, DSTREL_PAD, dtype=np.float32)
        ea_sl = np.zeros((nslot, EDGE_DIM), dtype=np.float32)
        for tl in range(TPC):
            tg = c * TPC + tl
            a, b = tile_starts[tg], tile_ends[tg]
            if b == a:
                continue
            s0 = off[tl] * P
            src_sl[s0:s0 + b - a] = src_s[a:b]
            drel_sl[s0:s0 + b - a] = (dst_s[a:b] - tg * P).astype(np.float32)
            ea_sl[s0:s0 + b - a] = ea_s[a:b]
        cores.append(dict(
            srcT=np.ascontiguousarray(src_sl.reshape(NCH, P).T),
            dstrelT=np.ascontiguousarray(drel_sl.reshape(NCH, P).T),
            eaT=np.ascontiguousarray(ea_sl.T).astype(ml_dtypes.bfloat16),
        ))
    return cores, Cloc, off, NCH


# ------------------------------------------------------- walrus wait legalize
def _legalize_waits(nc):
    import concourse.mybir as mybir
    k = 0
    for bb in nc.main_func.blocks:
        il = bb.instructions
        new = []
        for ins in il:
            si = ins.sync_info
            if si is not None and len(si.on_wait) > 1:
                waits = list(si.on_wait)
                for w in waits[:-1]:
                    nop = mybir.InstNoOp(name=f"wn{k}-{ins.name}", ins=[], outs=[])
                    k += 1
                    nop.engine = ins.engine
                    nop.sync_info = mybir.SyncInfo(on_wait=[w], on_update=[])
                    new.append(nop)
                ins.sync_info = mybir.SyncInfo(on_wait=[waits[-1]],
                                               on_update=list(si.on_update))
            new.append(ins)
        il[:] = new


# ------------------------------------------------------------- device program
def _build(Cloc, off, NCH):
    import concourse.bass as bass
    import concourse.mybir as mybir
    import concourse.tile as tile
    f32 = mybir.dt.float32
    bf16 = mybir.dt.bfloat16
    i32 = mybir.dt.int32
    Alu = mybir.AluOpType
    Act = mybir.ActivationFunctionType

    nc = bass.Bass()
    dp = nc.declare_dram_parameter
    x_fullT = dp("x_fullT", [NODE_DIM, NP_], bf16, isOutput=False)
    x_localT = dp("x_localT", [NODE_DIM, NLC], bf16, isOutput=False)
    srcT = dp("srcT", [P, NCH], i32, isOutput=False)
    dstrelT = dp("dstrelT", [P, NCH], f32, isOutput=False)
    eaT = dp("eaT", [EDGE_DIM, NCH * P], bf16, isOutput=False)
    iota_in = dp("iota_in", [P, P], f32, isOutput=False)
    ident_in = dp("ident_in", [P, P], bf16, isOutput=False)
    Wqkv1 = dp("Wqkv1", [NODE_DIM, 3 * HID], bf16, isOutput=False)
    We1 = dp("We1", [EDGE_DIM, HID], bf16, isOutput=False)
    Ws1 = dp("Ws1", [NODE_DIM, HID], bf16, isOutput=False)
    Wqkv2 = dp("Wqkv2", [HID, 3 * HID], bf16, isOutput=False)
    We2 = dp("We2", [EDGE_DIM, HID], bf16, isOutput=False)
    Ws2 = dp("Ws2", [HID, HID], bf16, isOutput=False)
    out = dp("out", [NLC, HID], f32, isOutput=True)

    # kv rows stored as raw 512B; declared f32 so the indirect gather walks
    # 128 4-byte elements per row instead of 256 2-byte ones
    kv_tab = nc.dram_tensor("kv_tab", [NP_, HID], f32, kind="Internal")
    q_loc = nc.dram_tensor("q_loc", [NLC, HID], bf16, kind="Internal")
    hT_loc = nc.dram_tensor("hT_loc", [HID, NLC], bf16, kind="Internal")
    kv2_loc = nc.dram_tensor("kv2_loc", [NLC, HID], f32, kind="Internal")

    # ---------------- phase A: q for local tiles, k|v table for all tiles
    # 4-tile batches: one load DMA, 4 matmuls, one evacuation, one store DMA.
    BA = 4

    def _qkv_loops(tc, cst, pool, psp, wt, qsrcT, kvsrc_slice):
        for i0 in range(0, TPC, BA):
            nb = min(BA, TPC - i0)
            xt = pool.tile([P, BA * P], bf16, tag="xl")
            nc.sync.dma_start(out=xt[:, 0:nb * P],
                              in_=qsrcT(i0, nb))
            ps = psp.tile([P, BA * HID], f32, space="PSUM", tag="pq")
            for j in range(nb):
                nc.tensor.matmul(out=ps[:, j * HID:(j + 1) * HID],
                                 lhsT=xt[:, j * P:(j + 1) * P],
                                 rhs=wt[:, 0:HID], start=True, stop=True)
            ev = pool.tile([P, BA * HID], bf16, tag="evq")
            nc.scalar.activation(out=ev[:, 0:nb * HID], in_=ps[:, 0:nb * HID],
                                 func=Act.Copy)
            nc.sync.dma_start(
                out=q_loc[i0 * P:(i0 + nb) * P, :].rearrange(
                    "(j p) d -> p j d", j=nb),
                in_=ev[:, 0:nb * HID].rearrange("p (j d) -> p j d", j=nb))
        for i0 in range(0, TILES, BA):
            nb = min(BA, TILES - i0)
            xt = pool.tile([P, BA * P], bf16, tag="xf")
            nc.sync.dma_start(out=xt[:, 0:nb * P], in_=kvsrc_slice(i0, nb))
            ps = psp.tile([P, BA * 2 * HID], f32, space="PSUM", tag="pkv")
            for j in range(nb):
                nc.tensor.matmul(out=ps[:, j * 2 * HID:(j + 1) * 2 * HID],
                                 lhsT=xt[:, j * P:(j + 1) * P],
                                 rhs=wt[:, HID:3 * HID], start=True, stop=True)
            ev = pool.tile([P, BA * 2 * HID], bf16, tag="evkv")
            nc.scalar.activation(out=ev[:, 0:nb * 2 * HID],
                                 in_=ps[:, 0:nb * 2 * HID], func=Act.Copy)
            nc.sync.dma_start(
                out=kv_tab[i0 * P:(i0 + nb) * P, :].rearrange(
                    "(j p) d -> p j d", j=nb),
                in_=ev[:].bitcast(f32)[:, 0:nb * HID].rearrange(
                    "p (j d) -> p j d", j=nb))

    def qkv_phase(locT, fullT, wqkv):
        with tile.TileContext(nc) as tc:
            with tc.tile_pool(name="qa_c", bufs=1) as cst, \
                 tc.tile_pool(name="qa_s", bufs=3) as pool, \
                 tc.tile_pool(name="qa_p", bufs=2, space="PSUM") as psp:
                wt = cst.tile([NODE_DIM, 3 * HID], bf16)
                nc.sync.dma_start(out=wt[:], in_=wqkv[:])
                _qkv_loops(tc, cst, pool, psp, wt,
                           lambda i0, nb: locT[:, i0 * P:(i0 + nb) * P],
                           lambda i0, nb: fullT[:, i0 * P:(i0 + nb) * P])

    # ---------------- edge phase
    def edge_phase(we, ws, xlocT, heads, relu, allgather_first=False):
        D = HID // heads
        scale = 1.0 / float(np.sqrt(D))
        W = HID + heads
        NMAX = int(Cloc.max())
        SB = 4                         # tiles per batched slab load
        NMAX4 = int(max(Cloc[t0:t0 + SB].sum()
                        for t0 in range(0, TPC, SB)))
        with tile.TileContext(nc) as tc:
            with tc.tile_pool(name="eg_c", bufs=1) as cst, \
                 tc.tile_pool(name="eg_sl", bufs=3) as slp, \
                 tc.tile_pool(name="eg_g", bufs=4) as gp, \
                 tc.tile_pool(name="eg_w", bufs=2) as wp, \
                 tc.tile_pool(name="eg_n", bufs=2) as npool, \
                 tc.tile_pool(name="eg_ps", bufs=3, space="PSUM") as pstage, \
                 tc.tile_pool(name="eg_pt", bufs=2, space="PSUM") as pst, \
                 tc.tile_pool(name="eg_pa", bufs=2, space="PSUM") as psacc:
                iota_f = cst.tile([P, P], f32)
                nc.sync.dma_start(out=iota_f[:], in_=iota_in[:])
                idt = cst.tile([P, P], bf16)
                nc.sync.dma_start(out=idt[:], in_=ident_in[:])
                wet = cst.tile([EDGE_DIM, HID], bf16)
                nc.sync.dma_start(out=wet[:], in_=we[:])
                wst = cst.tile([HID, HID], bf16)
                nc.sync.dma_start(out=wst[:], in_=ws[:])
                wq2 = wkv2 = None
                if relu:
                    wq2 = cst.tile([HID, HID], bf16)
                    nc.sync.dma_start(out=wq2[:], in_=Wqkv2[:, 0:HID])
                    wkv2 = cst.tile([HID, 2 * HID], bf16)
                    nc.sync.dma_start(out=wkv2[:], in_=Wqkv2[:, HID:3 * HID])

                for tl in range(TPC):
                    nch = int(Cloc[tl])
                    c0 = int(off[tl])
                    FD = nch * P
                    ngr = (nch + EGRP - 1) // EGRP

                    if tl % SB == 0:
                        t0b = tl
                        nch4 = int(Cloc[t0b:t0b + SB].sum())
                        cb0 = int(off[t0b])
                        ssl4 = slp.tile([P, NMAX4], i32, tag="ssl")
                        nc.sync.dma_start(out=ssl4[:, 0:nch4],
                                          in_=srcT[:, cb0:cb0 + nch4])
                        dsl4 = slp.tile([P, NMAX4], f32, tag="dsl")
                        nc.sync.dma_start(out=dsl4[:, 0:nch4],
                                          in_=dstrelT[:, cb0:cb0 + nch4])
                        nt4 = min(SB, TPC - t0b)
                        qt4 = slp.tile([P, SB * HID], bf16, tag="qt")
                        nc.sync.dma_start(
                            out=qt4[:, 0:nt4 * HID].rearrange(
                                "p (j d) -> p j d", j=nt4),
                            in_=q_loc[t0b * P:(t0b + nt4) * P, :].rearrange(
                                "(j p) d -> p j d", j=nt4))
                        xsk4 = slp.tile([P, SB * P], bf16, tag="xsk")
                        nc.sync.dma_start(out=xsk4[:, 0:nt4 * P],
                                          in_=xlocT[:, t0b * P:(t0b + nt4) * P])
                    sb_off = int(off[tl]) - int(off[t0b])
                    ssl = ssl4[:, sb_off:sb_off + nch]
                    dsl = dsl4[:, sb_off:sb_off + nch]
                    qtile = qt4[:, (tl - t0b) * HID:(tl - t0b + 1) * HID]
                    xsk = xsk4[:, (tl - t0b) * P:(tl - t0b + 1) * P]
                    ea_all = slp.tile([EDGE_DIM, NMAX * P], bf16, tag="ea")
                    nc.sync.dma_start(out=ea_all[:, 0:FD],
                                      in_=eaT[:, c0 * P:(c0 + nch) * P])

                    # merged k|v gathers, one per chunk (f32-typed raw rows)
                    kvg_r = gp.tile([P, NMAX * HID], f32, tag="kvg")
                    for k in range(nch):
                        nc.gpsimd.indirect_dma_start(
                            out=kvg_r[:, k * HID:(k + 1) * HID],
                            out_offset=None, in_=kv_tab[:],
                            in_offset=bass.IndirectOffsetOnAxis(
                                ap=ssl[:, k:k + 1], axis=0))
                    kvg = kvg_r[:].bitcast(bf16)

                    # S one-hot [slot, (j, c)] batched
                    S_all = wp.tile([P, NMAX * P], bf16, tag="S")
                    nc.vector.tensor_tensor(
                        out=S_all[:, 0:FD].rearrange("p (j c) -> p j c", j=nch),
                        in0=dsl.unsqueeze(2).to_broadcast([P, nch, P]),
                        in1=iota_f[:].unsqueeze(1).to_broadcast([P, nch, P]),
                        op=Alu.is_equal)

                    # eps / ST / qg staged through PSUM in EGRP-chunk groups
                    eps_sb = wp.tile([P, NMAX * HID], bf16, tag="eps")
                    st_sb = wp.tile([P, NMAX * P], bf16, tag="st")
                    qg_sb = wp.tile([P, NMAX * HID], bf16, tag="qg")
                    for g in range(ngr):
                        k0, k1 = g * EGRP, min(nch, (g + 1) * EGRP)
                        nk = k1 - k0
                        pe = pstage.tile([P, EGRP * HID], f32, space="PSUM",
                                         tag="stage")
                        for k in range(k0, k1):
                            j = k - k0
                            nc.tensor.matmul(
                                out=pe[:, j * HID:(j + 1) * HID],
                                lhsT=ea_all[:, k * P:(k + 1) * P],
                                rhs=wet[:], start=True, stop=True)
                        nc.scalar.activation(out=eps_sb[:, k0 * HID:k1 * HID],
                                             in_=pe[:, 0:nk * HID], func=Act.Copy)
                        pt = pst.tile([P, EGRP * P], bf16, space="PSUM",
                                      tag="staget")
                        for k in range(k0, k1):
                            j = k - k0
                            nc.tensor.transpose(
                                out=pt[:, j * P:(j + 1) * P],
                                in_=S_all[:, k * P:(k + 1) * P], identity=idt[:])
                        nc.scalar.activation(out=st_sb[:, k0 * P:k1 * P],
                                             in_=pt[:, 0:nk * P], func=Act.Copy)
                        pq = pstage.tile([P, EGRP * HID], f32, space="PSUM",
                                         tag="stage")
                        for k in range(k0, k1):
                            j = k - k0
                            nc.tensor.matmul(
                                out=pq[:, j * HID:(j + 1) * HID],
                                lhsT=st_sb[:, k * P:(k + 1) * P],
                                rhs=qtile, start=True, stop=True)
                        nc.scalar.activation(out=qg_sb[:, k0 * HID:k1 * HID],
                                             in_=pq[:, 0:nk * HID], func=Act.Copy)

                    # batched DVE: kj, vj, prod, alpha
                    kj = wp.tile([P, NMAX * HID], bf16, tag="kj")
                    nc.vector.tensor_tensor(
                        out=kj[:, 0:FD].rearrange("p (j d) -> p j d", j=nch),
                        in0=kvg[:, 0:nch * 2 * HID].rearrange(
                            "p (j d) -> p j d", j=nch)[:, :, 0:HID],
                        in1=eps_sb[:, 0:FD].rearrange("p (j d) -> p j d", j=nch),
                        op=Alu.add)
                    vj = wp.tile([P, NMAX * HID], bf16, tag="vj")
                    nc.vector.tensor_tensor(
                        out=vj[:, 0:FD].rearrange("p (j d) -> p j d", j=nch),
                        in0=kvg[:, 0:nch * 2 * HID].rearrange(
                            "p (j d) -> p j d", j=nch)[:, :, HID:2 * HID],
                        in1=eps_sb[:, 0:FD].rearrange("p (j d) -> p j d", j=nch),
                        op=Alu.add)
                    prod = wp.tile([P, NMAX * HID], bf16, tag="prod")
                    nc.vector.tensor_tensor(out=prod[:, 0:FD], in0=kj[:, 0:FD],
                                            in1=qg_sb[:, 0:FD], op=Alu.mult)
                    alpha = wp.tile([P, NMAX * 8], bf16, tag="alpha")
                    with nc.allow_low_precision(reason="attention logits are O(1); "
                                                "bf16 sum of 16 terms is fine"):
                        nc.vector.tensor_reduce(
                            out=alpha[:, 0:nch * heads],
                            in_=prod[:, 0:FD].rearrange("p (g d) -> p g d", d=D),
                            axis=mybir.AxisListType.X, op=Alu.add)

                    # rhs = [vj*exp | exp]
                    rhs = wp.tile([P, NMAX * W], bf16, tag="rhs")
                    rhs3 = rhs[:, 0:nch * W].rearrange("p (j w) -> p j w", j=nch)
                    nc.scalar.activation(
                        out=rhs3[:, :, HID:W],
                        in_=alpha[:, 0:nch * heads].rearrange(
                            "p (j h) -> p j h", j=nch),
                        func=Act.Exp, scale=scale)
                    nc.vector.tensor_tensor(
                        out=rhs3[:, :, 0:HID].rearrange(
                            "p j (h d) -> p j h d", h=heads),
                        in0=vj[:, 0:FD].rearrange(
                            "p (j h d) -> p j h d", j=nch, h=heads),
                        in1=rhs3[:, :, HID:W].unsqueeze(3).to_broadcast(
                            [P, nch, heads, D]),
                        op=Alu.mult)

                    # segment sum via one-hot matmul, accumulated per tile
                    acc = psacc.tile([P, W], f32, space="PSUM", tag="acc")
                    for k in range(nch):
                        nc.tensor.matmul(
                            out=acc[:], lhsT=S_all[:, k * P:(k + 1) * P],
                            rhs=rhs[:, k * W:(k + 1) * W],
                            start=(k == 0), stop=(k == nch - 1))

                    # ---- node update
                    sb_t = npool.tile([P, heads], f32, tag="sb")
                    nc.vector.tensor_scalar_add(out=sb_t[:],
                                                in0=acc[:, HID:W], scalar1=1e-16)
                    rinv = npool.tile([P, heads], f32, tag="rinv")
                    nc.vector.reciprocal(out=rinv[:], in_=sb_t[:])
                    attn = npool.tile([P, HID], f32, tag="attn")
                    nc.vector.tensor_tensor(
                        out=attn[:].rearrange("p (h d) -> p h d", h=heads),
                        in0=acc[:, 0:HID].rearrange("p (h d) -> p h d", h=heads),
                        in1=rinv[:].unsqueeze(2).to_broadcast([P, heads, D]),
                        op=Alu.mult)
                    skt = pstage.tile([P, EGRP * HID], f32, space="PSUM",
                                      tag="stage")
                    sk = skt[:, 0:HID]
                    nc.tensor.matmul(out=sk, lhsT=xsk, rhs=wst[:],
                                     start=True, stop=True)
                    ht = npool.tile([P, HID], f32, tag="ht")
                    nc.vector.tensor_tensor(out=ht[:], in0=attn[:], in1=sk,
                                            op=Alu.add)
                    if relu:
                        ht2 = npool.tile([P, HID], bf16, tag="ht2")
                        nc.scalar.activation(out=ht2[:], in_=ht[:], func=Act.Lrelu,
                                             alpha=0.01)
                        tpt = pst.tile([P, EGRP * P], bf16, space="PSUM",
                                       tag="staget")
                        tp = tpt[:, 0:P]
                        nc.tensor.transpose(out=tp, in_=ht2[:], identity=idt[:])
                        hTt = npool.tile([P, P], bf16, tag="hTt")
                        nc.scalar.activation(out=hTt[:], in_=tp, func=Act.Copy)
                        nc.sync.dma_start(out=hT_loc[:, tl * P:(tl + 1) * P],
                                              in_=hTt[:])
                        # fused layer-2 q|k|v for this tile (hides the whole
                        # layer-2 phase A; kv2 gets AllGathered between layers)
                        q2t = pstage.tile([P, EGRP * HID], f32, space="PSUM",
                                          tag="stage")
                        nc.tensor.matmul(out=q2t[:, 0:HID], lhsT=hTt[:],
                                         rhs=wq2[:], start=True, stop=True)
                        q2e = npool.tile([P, HID], bf16, tag="q2e")
                        nc.scalar.activation(out=q2e[:], in_=q2t[:, 0:HID],
                                             func=Act.Copy)
                        nc.sync.dma_start(out=q_loc[tl * P:(tl + 1) * P, :],
                                              in_=q2e[:])
                        kv2t = pstage.tile([P, EGRP * HID], f32, space="PSUM",
                                           tag="stage")
                        nc.tensor.matmul(out=kv2t[:, 0:2 * HID], lhsT=hTt[:],
                                         rhs=wkv2[:], start=True, stop=True)
                        kv2e = npool.tile([P, 2 * HID], bf16, tag="kv2e")
                        nc.scalar.activation(out=kv2e[:], in_=kv2t[:, 0:2 * HID],
                                             func=Act.Copy)
                        nc.sync.dma_start(
                            out=kv2_loc[tl * P:(tl + 1) * P, :],
                            in_=kv2e[:].bitcast(f32))
                    else:
                        nc.sync.dma_start(out=out[tl * P:(tl + 1) * P, :],
                                            in_=ht[:])

                if relu:
                    nc.gpsimd.collective_compute(
                        "AllGather", Alu.bypass,
                        replica_groups=[list(range(NCORES))],
                        ins=[kv2_loc[:].opt()], outs=[kv_tab[:].opt()])


    qkv_phase(x_localT, x_fullT, Wqkv1)
    edge_phase(We1, Ws1, x_localT, heads=8, relu=True, allgather_first=False)
    edge_phase(We2, Ws2, hT_loc, heads=1, relu=False, allgather_first=True)

    _legalize_waits(nc)
    return nc


_CACHE = {}


def kernel(x, ei, ea, Wq1, bq1, Wk1, bk1, Wv1, bv1, We1, Ws1, bs1,
           Wq2, bq2, Wk2, bk2, Wv2, bv2, We2, Ws2, bs2):
    import ml_dtypes
    from concourse.bass_utils import run_bass_kernel_spmd
    bf = ml_dtypes.bfloat16

    for b in (bq1, bk1, bv1, bs1, bq2, bk2, bv2, bs2):
        assert not np.any(np.asarray(b)), "nonzero biases not supported"

    x = np.asarray(x, np.float32)
    x_pad = np.zeros((NP_, NODE_DIM), np.float32)
    x_pad[:N] = x
    x_fullT = np.ascontiguousarray(x_pad.T).astype(bf)
    cores, Cloc, off, NCH = _prep(np.asarray(ei), np.asarray(ea))

    key = (NCH, tuple(Cloc))
    if key not in _CACHE:
        _CACHE[key] = _build(Cloc, off, NCH)
    nc = _CACHE[key]

    def cat3(a, b, c):
        return np.ascontiguousarray(np.concatenate(
            [np.asarray(a, np.float32), np.asarray(b, np.float32),
             np.asarray(c, np.float32)], axis=1)).astype(bf)

    Wqkv1 = cat3(Wq1, Wk1, Wv1)
    Wqkv2 = cat3(Wq2, Wk2, Wv2)
    iota_in = np.tile(np.arange(P, dtype=np.float32)[None, :], (P, 1))
    ident_in = np.eye(P, dtype=np.float32).astype(bf)

    in_maps = []
    for c in range(NCORES):
        pc = cores[c]
        in_maps.append({
            "x_fullT": x_fullT,
            "x_localT": np.ascontiguousarray(
                x_fullT[:, c * NLC:(c + 1) * NLC]),
            "srcT": pc["srcT"], "dstrelT": pc["dstrelT"], "eaT": pc["eaT"],
            "iota_in": iota_in, "ident_in": ident_in,
            "Wqkv1": Wqkv1, "We1": np.asarray(We1, np.float32).astype(bf),
            "Ws1": np.asarray(Ws1, np.float32).astype(bf),
            "Wqkv2": Wqkv2, "We2": np.asarray(We2, np.float32).astype(bf),
            "Ws2": np.asarray(Ws2, np.float32).astype(bf),
        })
    res = run_bass_kernel_spmd(nc, in_maps, list(range(NCORES)))
    global LAST_RESULT
    LAST_RESULT = res
    out = np.concatenate([res.results[c]["out"] for c in range(NCORES)], axis=0)
    return np.ascontiguousarray(out[:N])


LAST_RESULT = None
